# revision 2
# baseline (speedup 1.0000x reference)
"""Chamfer distance v2: centers-in-partitions layout, fp16 DVE, PE broadcast.

Per core, per batch b:
  t_row   [1, 9600]   core's target shard (DRAM -> partition 0, chunked)
  PE:     psum[p, f] = t[f]          (ones[1,128]^T @ t_row chunk, K=1)
  ScalarE: d2T[p, f] = (128*t[f] - 128*c[ct*128+p])^2   fp16, in [~1e-6, 16384]
           (scale=128, bias=-128*c from transposed edge DMA; ct in {0,1})
  dir1 (per-center min over targets): fp16 tt-min tree along free + reduce
  dir2 (per-target min over centers): M2 = min(d2T_ct0, d2T_ct1) [128, 9600]
        -> dma_start_transpose -> T [128, 75, 128] -> tt-min tree + reduce
        -> per-target mins [128, 75] -> sum -> [128, 1]
Host: min over cores for dir1, sums, /16384, mean over batches.
"""

import sys

if "/opt/trn_rl_repo" not in sys.path:
    sys.path.insert(0, "/opt/trn_rl_repo")

import numpy as np

import concourse.bass as bass
import concourse.tile as tile
from concourse import bacc, mybir
from concourse.bass_utils import run_bass_kernel_spmd

B = 2
N = 76800
E = 257
K = 256
NCORES = 8
NSH = N // NCORES   # 9600
P = 128
CHUNK = 1920        # targets per group
NG = NSH // CHUNK   # 5 groups
NBLK = NSH // P     # 75 transpose blocks
SCALE = 128.0       # fp16 range scaling; d2 is scaled by SCALE^2 = 16384

F32 = mybir.dt.float32
F16 = mybir.dt.float16
MIN = mybir.AluOpType.min
ADD = mybir.AluOpType.add
AX = mybir.AxisListType


def _build_kernel(nc, tc, t_in, e_in, dir1_out, dir2_out):
    from contextlib import ExitStack

    ctx = ExitStack()
    const_pool = ctx.enter_context(tc.tile_pool(name="const", bufs=2))
    trow_pool = ctx.enter_context(tc.tile_pool(name="trow", bufs=3))
    psum_pool = ctx.enter_context(tc.tile_pool(name="psum", bufs=2, space="PSUM"))
    d2_pool = ctx.enter_context(tc.tile_pool(name="d2", bufs=2))
    tree_pool = ctx.enter_context(tc.tile_pool(name="tree", bufs=2))
    acc_pool = ctx.enter_context(tc.tile_pool(name="acc", bufs=1))
    out_pool = ctx.enter_context(tc.tile_pool(name="out", bufs=2))

    ones = const_pool.tile([1, P], F32, tag="ones")
    nc.vector.memset(ones[:], 1.0)

    m2 = {}
    for b in range(B):
        # -64*(e[j] + e[j+1]) = -128*c_j, laid out [p, ct] for ct*128+p
        ec0 = const_pool.tile([P, 2], F32, tag="ec0")
        nc.sync.dma_start(ec0[:], e_in[b, 0:K].rearrange("(k p) -> p k", p=P))
        ec1 = const_pool.tile([P, 2], F32, tag="ec1")
        nc.sync.dma_start(ec1[:], e_in[b, 1 : K + 1].rearrange("(k p) -> p k", p=P))
        esum = const_pool.tile([P, 2], F32, tag="esum")
        nc.vector.tensor_add(esum[:], ec0[:], ec1[:])
        negc = const_pool.tile([P, 2], F32, tag="negc")
        nc.vector.tensor_scalar_mul(negc[:], esum[:], -64.0)

        m2[b] = acc_pool.tile([P, NSH], F16, tag=f"m2_{b}", name=f"m2_{b}")
        # per-(ct) chunk mins after 3 tree levels: [P, NG, CHUNK//8]
        cmins = [
            acc_pool.tile(
                [P, NG, CHUNK // 8], F16, tag=f"cm_{b}_{ct}", name=f"cm_{b}_{ct}"
            )
            for ct in range(2)
        ]

        for g in range(NG):
            trowg = trow_pool.tile([1, CHUNK], F32, tag="trowg")
            nc.sync.dma_start(
                trowg[:], t_in[b, g * CHUNK : (g + 1) * CHUNK].unsqueeze(0)
            )
            tb = psum_pool.tile([P, CHUNK], F32, tag="tb")
            for k in range(0, CHUNK, 512):
                w = min(512, CHUNK - k)
                nc.tensor.matmul(
                    tb[:, k : k + w], ones[:], trowg[:, k : k + w],
                    start=True, stop=True,
                )
            d2t = [None, None]
            for ct in range(2):
                d2t[ct] = d2_pool.tile([P, CHUNK], F16, tag=f"d2t{ct}", name=f"d2t{ct}")
                nc.scalar.activation(
                    d2t[ct][:], tb[:],
                    mybir.ActivationFunctionType.Square,
                    bias=negc[:, ct : ct + 1],
                    scale=SCALE,
                )
                # dir1 tree: CHUNK -> CHUNK/8 via fp16 2x tt-min
                h = CHUNK // 2
                l1 = tree_pool.tile([P, h], F16, tag="l1")
                nc.vector.tensor_tensor(
                    l1[:], d2t[ct][:, 0:h], d2t[ct][:, h : 2 * h], op=MIN
                )
                h //= 2
                l2 = tree_pool.tile([P, h], F16, tag="l2")
                nc.vector.tensor_tensor(l2[:], l1[:, 0:h], l1[:, h : 2 * h], op=MIN)
                h //= 2
                nc.vector.tensor_tensor(
                    cmins[ct][:, g, :], l2[:, 0:h], l2[:, h : 2 * h], op=MIN
                )
            # dir2: fold the two center halves
            nc.vector.tensor_tensor(
                m2[b][:, g * CHUNK : (g + 1) * CHUNK], d2t[0][:], d2t[1][:], op=MIN
            )

        # dir1 finals: [P, NG, CHUNK//8] -> [P, 1] (scaled d2 min, fp32 out)
        for ct in range(2):
            d1fin = out_pool.tile([P, 1], F32, tag="d1fin")
            nc.vector.tensor_reduce(
                out=d1fin[:], in_=cmins[ct][:], op=MIN, axis=AX.XY
            )
            nc.sync.dma_start(dir1_out[b, ct].unsqueeze(1), d1fin[:])

    # dir2: transpose + per-target min + sum (after all SBUF-copy DMA traffic)
    for b in range(B):
        tt = acc_pool.tile([P, NBLK, P], F16, tag=f"tt_{b}")
        nc.sync.dma_start_transpose(tt[:], m2[b][:])
        # tree over the innermost 128 (center pairs)
        h = 64
        u1 = tree_pool.tile([P, NBLK, h], F16, tag="u1")
        nc.vector.tensor_tensor(
            u1[:], tt[:, :, 0:h], tt[:, :, h : 2 * h], op=MIN
        )
        h //= 2
        u2 = tree_pool.tile([P, NBLK, h], F16, tag="u2")
        nc.vector.tensor_tensor(u2[:], u1[:, :, 0:h], u1[:, :, h : 2 * h], op=MIN)
        h //= 2
        u3 = tree_pool.tile([P, NBLK, h], F16, tag="u3")
        nc.vector.tensor_tensor(u3[:], u2[:, :, 0:h], u2[:, :, h : 2 * h], op=MIN)
        tmin = tree_pool.tile([P, NBLK], F16, tag="tmin")
        nc.vector.tensor_reduce(out=tmin[:], in_=u3[:], op=MIN, axis=AX.X)
        d2sum = out_pool.tile([P, 1], F32, tag="d2sum")
        nc.vector.tensor_reduce(out=d2sum[:], in_=tmin[:], op=ADD, axis=AX.X)
        nc.sync.dma_start(dir2_out[b], d2sum[:])

    ctx.close()


_CACHE = {}


def _get_compiled():
    if "nc" in _CACHE:
        return _CACHE["nc"]
    nc = bacc.Bacc(
        "TRN2",
        target_bir_lowering=False,
        debug=False,
        enable_asserts=False,
        num_devices=NCORES,
    )
    t_in = nc.dram_tensor("t", [B, NSH], F32, kind="ExternalInput").ap()
    e_in = nc.dram_tensor("edges", [B, E], F32, kind="ExternalInput").ap()
    dir1_out = nc.dram_tensor("dir1", [B, 2, P], F32, kind="ExternalOutput").ap()
    dir2_out = nc.dram_tensor("dir2", [B, P, 1], F32, kind="ExternalOutput").ap()

    with tile.TileContext(nc) as tc:
        _build_kernel(nc, tc, t_in, e_in, dir1_out, dir2_out)
    nc.compile()
    _CACHE["nc"] = nc
    return nc


def kernel(target: np.ndarray, bin_edges: np.ndarray) -> np.ndarray:
    target = np.asarray(target, dtype=np.float32)
    bin_edges = np.asarray(bin_edges, dtype=np.float32)

    t_flat = target.reshape(B, N)
    in_maps = []
    for c in range(NCORES):
        shard = t_flat[:, c * NSH : (c + 1) * NSH]
        in_maps.append({"t": np.ascontiguousarray(shard), "edges": bin_edges})

    nc = _get_compiled()
    res = run_bass_kernel_spmd(nc, in_maps, list(range(NCORES))).results

    dir1 = np.stack([r["dir1"] for r in res])  # [NCORES, B, 2, P] scaled
    dir2 = np.stack([r["dir2"] for r in res])  # [NCORES, B, P, 1] scaled

    per_center = dir1.min(axis=0).reshape(B, K)          # [B, 256]
    d1 = per_center.sum(axis=1, dtype=np.float64) / (SCALE * SCALE)
    d2 = dir2.sum(axis=(0, 2, 3), dtype=np.float64) / (SCALE * SCALE)
    out = np.float32((d1 + d2).mean())
    return np.asarray(out, dtype=np.float32)


# revision 3
# speedup vs baseline: 1.0848x; 1.0848x over previous
"""Chamfer distance v3: DMA broadcast, fused ct tiles, per-chunk transposes.

Per core, per batch b, per chunk g (3 chunks of 3200 targets):
  t_bcast [128, 3200] f32   <- DRAM broadcast DMA (gpsimd/SWDGE)
  ScalarE: d2both[:, ct, :] = (128*t - 128*c[ct*128+p])^2  fp16 (scale=128)
  DVE dir1 tree (both ct at once): 3 levels -> cmins[P, 2, g, 400]
  DVE m2 chunk = min(ct0, ct1) [128, 3200]
  nc.sync dma_start_transpose -> T [128, 25, 128]
  DVE dir2 tree over innermost 128 -> tmin[:, g*25:(g+1)*25]
Finals per batch: dir1 reduce -> [P,2] -> DRAM; dir2 sum -> [P,1] -> DRAM.
Host: min over cores for dir1, sums, /16384, mean over batches.
"""

import sys

if "/opt/trn_rl_repo" not in sys.path:
    sys.path.insert(0, "/opt/trn_rl_repo")

import numpy as np

import concourse.bass as bass
import concourse.tile as tile
from concourse import bacc, mybir
from concourse.bass_utils import run_bass_kernel_spmd

B = 2
N = 76800
E = 257
K = 256
NCORES = 8
NSH = N // NCORES   # 9600
P = 128
CHUNK = 3200
NG = NSH // CHUNK   # 3
NBLK = NSH // P     # 75 (25 per chunk)
CBLK = CHUNK // P   # 25
SCALE = 128.0       # d2 carried scaled by SCALE^2 = 16384

F32 = mybir.dt.float32
F16 = mybir.dt.float16
MIN = mybir.AluOpType.min
ADD = mybir.AluOpType.add
AX = mybir.AxisListType


def _build_kernel(nc, tc, t_in, e_in, dir1_out, dir2_out):
    from contextlib import ExitStack

    ctx = ExitStack()
    const_pool = ctx.enter_context(tc.tile_pool(name="const", bufs=2))
    tb_pool = ctx.enter_context(tc.tile_pool(name="tb", bufs=2))
    d2_pool = ctx.enter_context(tc.tile_pool(name="d2", bufs=2))
    tree_pool = ctx.enter_context(tc.tile_pool(name="tree", bufs=2))
    m2_pool = ctx.enter_context(tc.tile_pool(name="m2", bufs=2))
    tp_pool = ctx.enter_context(tc.tile_pool(name="tp", bufs=2))
    acc_pool = ctx.enter_context(tc.tile_pool(name="acc", bufs=1))
    out_pool = ctx.enter_context(tc.tile_pool(name="out", bufs=2))

    for b in range(B):
        # -64*(e[j] + e[j+1]) = -128*c_j laid out [p, ct], center j = ct*128+p
        ec0 = const_pool.tile([P, 2], F32, tag="ec0")
        nc.sync.dma_start(ec0[:], e_in[b, 0:K].rearrange("(k p) -> p k", p=P))
        ec1 = const_pool.tile([P, 2], F32, tag="ec1")
        nc.sync.dma_start(ec1[:], e_in[b, 1 : K + 1].rearrange("(k p) -> p k", p=P))
        esum = const_pool.tile([P, 2], F32, tag="esum")
        nc.vector.tensor_add(esum[:], ec0[:], ec1[:])
        negc = const_pool.tile([P, 2], F32, tag="negc")
        nc.vector.tensor_scalar_mul(negc[:], esum[:], -64.0)

        cmins = acc_pool.tile(
            [P, 2, NG, CHUNK // 8], F16, tag=f"cm_{b}", name=f"cm_{b}"
        )
        tmin = acc_pool.tile([P, NBLK], F16, tag=f"tmin_{b}", name=f"tmin_{b}")

        for g in range(NG):
            tb = tb_pool.tile([P, CHUNK], F32, tag="tb")
            nc.gpsimd.dma_start(
                tb[:],
                t_in[b, g * CHUNK : (g + 1) * CHUNK]
                .unsqueeze(0)
                .to_broadcast((P, CHUNK)),
            )
            d2both = d2_pool.tile([P, 2, CHUNK], F16, tag="d2both")
            for ct in range(2):
                nc.scalar.activation(
                    d2both[:, ct, :], tb[:],
                    mybir.ActivationFunctionType.Square,
                    bias=negc[:, ct : ct + 1],
                    scale=SCALE,
                )
            # dir1 tree over targets, both ct lanes at once
            h = CHUNK // 2
            l1 = tree_pool.tile([P, 2, h], F16, tag="l1")
            nc.vector.tensor_tensor(
                l1[:], d2both[:, :, 0:h], d2both[:, :, h : 2 * h], op=MIN
            )
            h //= 2
            l2 = tree_pool.tile([P, 2, h], F16, tag="l2")
            nc.vector.tensor_tensor(
                l2[:], l1[:, :, 0:h], l1[:, :, h : 2 * h], op=MIN
            )
            h //= 2
            nc.vector.tensor_tensor(
                cmins[:, :, g, :], l2[:, :, 0:h], l2[:, :, h : 2 * h], op=MIN
            )
            # dir2: fold the two center halves, transpose, tree over 128 centers
            m2 = m2_pool.tile([P, CHUNK], F16, tag="m2")
            nc.vector.tensor_tensor(
                m2[:], d2both[:, 0, :], d2both[:, 1, :], op=MIN
            )
            tt = tp_pool.tile([P, CBLK, P], F16, tag="tt")
            nc.sync.dma_start_transpose(tt[:], m2[:])
            h = 64
            u1 = tree_pool.tile([P, CBLK, h], F16, tag="u1")
            nc.vector.tensor_tensor(
                u1[:], tt[:, :, 0:h], tt[:, :, h : 2 * h], op=MIN
            )
            h //= 2
            u2 = tree_pool.tile([P, CBLK, h], F16, tag="u2")
            nc.vector.tensor_tensor(
                u2[:], u1[:, :, 0:h], u1[:, :, h : 2 * h], op=MIN
            )
            h //= 2
            u3 = tree_pool.tile([P, CBLK, h], F16, tag="u3")
            nc.vector.tensor_tensor(
                u3[:], u2[:, :, 0:h], u2[:, :, h : 2 * h], op=MIN
            )
            nc.vector.tensor_reduce(
                out=tmin[:, g * CBLK : (g + 1) * CBLK], in_=u3[:], op=MIN, axis=AX.X
            )

        # dir1 final: [P, 2, NG, CHUNK//8] -> [P, 2] (scaled, fp32)
        d1fin = out_pool.tile([P, 2], F32, tag="d1fin")
        nc.vector.tensor_reduce(out=d1fin[:], in_=cmins[:], op=MIN, axis=AX.XY)
        nc.sync.dma_start(dir1_out[b].rearrange("c p -> p c"), d1fin[:])
        # dir2 final: sum of per-target mins
        d2sum = out_pool.tile([P, 1], F32, tag="d2sum")
        nc.vector.tensor_reduce(out=d2sum[:], in_=tmin[:], op=ADD, axis=AX.X)
        nc.sync.dma_start(dir2_out[b], d2sum[:])

    ctx.close()


_CACHE = {}


def _get_compiled():
    if "nc" in _CACHE:
        return _CACHE["nc"]
    nc = bacc.Bacc(
        "TRN2",
        target_bir_lowering=False,
        debug=False,
        enable_asserts=False,
        num_devices=NCORES,
    )
    t_in = nc.dram_tensor("t", [B, NSH], F32, kind="ExternalInput").ap()
    e_in = nc.dram_tensor("edges", [B, E], F32, kind="ExternalInput").ap()
    dir1_out = nc.dram_tensor("dir1", [B, 2, P], F32, kind="ExternalOutput").ap()
    dir2_out = nc.dram_tensor("dir2", [B, P, 1], F32, kind="ExternalOutput").ap()

    with tile.TileContext(nc) as tc:
        _build_kernel(nc, tc, t_in, e_in, dir1_out, dir2_out)
    nc.compile()
    _CACHE["nc"] = nc
    return nc


def kernel(target: np.ndarray, bin_edges: np.ndarray) -> np.ndarray:
    target = np.asarray(target, dtype=np.float32)
    bin_edges = np.asarray(bin_edges, dtype=np.float32)

    t_flat = target.reshape(B, N)
    in_maps = []
    for c in range(NCORES):
        shard = t_flat[:, c * NSH : (c + 1) * NSH]
        in_maps.append({"t": np.ascontiguousarray(shard), "edges": bin_edges})

    nc = _get_compiled()
    res = run_bass_kernel_spmd(nc, in_maps, list(range(NCORES))).results

    dir1 = np.stack([r["dir1"] for r in res])  # [NCORES, B, 2, P] scaled
    dir2 = np.stack([r["dir2"] for r in res])  # [NCORES, B, P, 1] scaled

    per_center = dir1.min(axis=0).reshape(B, K)
    d1 = per_center.sum(axis=1, dtype=np.float64) / (SCALE * SCALE)
    d2 = dir2.sum(axis=(0, 2, 3), dtype=np.float64) / (SCALE * SCALE)
    out = np.float32((d1 + d2).mean())
    return np.asarray(out, dtype=np.float32)
